# revision 48
# baseline (speedup 1.0000x reference)
"""Trainium2 Bass kernel for the CPC module (nn_CPCModule_63565515981073).

Data-parallel over batch: 64 sequences -> 8 NeuronCores x 8 sequences.
All parameters replicated; the scalar loss partials are summed on host.

Math note: the reference's `-log_softmax(logits)[..., arange]` is an identity
gather, so the per-k loss is mean_{b,t,j}( LSE_row - logits ) =
mean_{b,t}( LSE_{b,t} ) - mean_{b,t,j}( logits ). We compute, per (b,k):
  - rowwise LSE via max-shifted exp with the ACT engine's accumulator
  - the logits-sum term as predsum . zsum (sum over t of pred, sum over j of z)
1/TEMP is folded into Wp/bp on the host so logits come out of the PE scaled.

GRU scan: chunked-parallel with warm-up burn-in. The GRU map is contractive
(update gate ~0.5 per step), so a chunk started from h=0 at W steps before
its window converges to the true trajectory. End-to-end loss rel-err on the
reference inputs: 4.1e-4 / 2.0e-4 / 9.2e-5 for W = 4 / 5 / 6 (vs 4e-5 bf16
noise floor and the 2e-2 gate; all measured end-to-end on device). T=512 is
split into C=32 chunks of S=16 steps; all C chunks x Bl sequences advance
together as 256 lanes of one per-step instruction group, so wall time is
(S+W)=20 chain latencies instead of 512. Chunk 0 pads with z=0 input, which keeps h exactly 0 through warm-up
(needs bhn == 0; bhn != 0 falls back to C=1, W=0 = exact serial scan).

Per-step dependency chain (the wall, ~4.4us in the cost model):
aa -> 4 Wh r-matmuls -> sigmoid(r) [ACT] -> npd [DVE] -> nsm [DVE] ->
tanh [ACT] -> aa [DVE]. The h = (1-z)*n + z*h_prev update is split by
linearity of the next matmul: gh_{t+1} = Wh^T aa + Wh^T bzh with
aa = (1-z).n (on-chain, DVE) and bzh = z.h_prev (off-chain, POOL, ready
early), so the final add never sits on the chain; DVE materializes
h = aa + bzh straight into cT (strided view) for the loss phase. POOL also
pre-copies each step's strided gi_n window into a contiguous bf16 tile so
nsm gets the DVE 2x packed mode.

Loss phase (the throughput wall, ~330us): per (k,b): pred matmuls + one DVE
PSUM->SBUF copy (with predsum accum for the dot-term), 4 logits matmuls into
PSUM pairs, exact row-max via DVE reduce (subsampled maxes are NOT safe:
logits spike to +328 with 243-gap outliers; fp32 exp needs a shift within
+-87 of the true row max), then 4 ACT exps with per-partition bias and
accum_out. PSUM banks: pred bufs=1 (1) + lg bufs=3 (6) + es scratch (1) = 8;
the 3-deep logits-pair pipeline hides the reduce->exp sync, putting ACT and
DVE at ~96% busy (3.1us per (k,b) unit). The final 1x1 reduction matmul
aliases the es bank. POOL cannot read PSUM on TRN2, and walrus rejects
>512-col matmul moving operands and dual-PSUM-input DVE ops -- all three
shaped this assignment.

The encoder+projection pair is folded into a single matmul on the host
(z = x @ (W_enc W_proj) + (b_enc W_proj + b_proj), exact in real
arithmetic), deleting the encT stage entirely.

Layouts (partition dim first):
  xT    [128, 2, B, T]     bf16  x transposed: feature-on-partitions
  zT    [128, B, T]        bf16  folded enc+proj output transposed (P=128)
  gi    [128, 6, B, W+T]   bf16  z @ Wi + bi, transposed; W zero-pad cols
  cT    [128, B, 2, T]     bf16  GRU hidden states transposed, b-major
Scan state: [128, 2, C, B] lanes (chunk-major); hand-crafted strided APs
(_view) address the per-step (c, b) -> t = c*S + s - W positions of gi and
cT directly, so the overlapping chunk windows need no data duplication.

Cost-model timeline (TimelineSim): 442us/core vs 1611us for the previous
serial-scan kernel (3.65x). Startup: all constants ship in two packed DMAs
(per-DMA fixed overhead ~625ns on the single HWDGE queue); x is sliced per
(kb, sequence) so the z matmuls start after the first slice. NOTE: batching
the x DMA into [128, 2, 512] dual-sequence slices produced garbage on the
device despite a clean sim -- keep the per-sequence slicing.
"""

import math

import numpy as np
import ml_dtypes

import bass_rust
import concourse.bass as bass
import concourse.bacc as bacc
import concourse.mybir as mybir
import concourse.tile as tile
from concourse.bass_utils import run_bass_kernel_spmd

BF16 = mybir.dt.bfloat16
F32 = mybir.dt.float32
AF = mybir.ActivationFunctionType
ALU = mybir.AluOpType
AX = mybir.AxisListType

N_CORES = 8
B_TOT, T_FULL, F_IN = 64, 512, 256
ENC, P, H, K_FULL = 256, 128, 256, 12
TEMP = 0.1
CH_FE = 512

nbf = ml_dtypes.bfloat16


def _ceil_div(a, b):
    return (a + b - 1) // b


def _view(ap, dims, offset):
    """Hand-crafted strided view: keep the partition dim, replace free dims."""
    v = ap.copy()
    v.ap = bass_rust.VecI64Pair(
        [list(ap.ap[0])] + [[int(s), int(c)] for s, c in dims]
    )
    v.offset = int(ap.offset + offset)
    return v


def build_kernel(Bl, T, K, C=32, W=4, debug=False, bhn_zero=True, split_scan=False):
    """Build the Bass program for one core with Bl local sequences."""
    if not bhn_zero:
        C, W = 1, 0  # exact serial scan fallback
    S = T // C
    assert S * C == T
    nc = bacc.Bacc("TRN2", target_bir_lowering=False, debug=False)
    NT = Bl * T
    TP = T + W
    L = C * Bl          # scan lanes per instruction
    NS = S + W          # scan steps
    if split_scan:
        scan_groups = [(0, Bl // 2), (Bl // 2, Bl - Bl // 2)]
    else:
        scan_groups = [(0, Bl)]
    n_m = _ceil_div(T - 1, 128)  # logits row-tiles per (b,k)
    assert n_m == _ceil_div(T - K, 128), "per-k mtile count must be constant"
    dbg = {}
    if debug:
        dbg["zT"] = nc.dram_tensor("dbg_zT", [128, Bl, T], BF16, kind="ExternalOutput")
        dbg["gi"] = nc.dram_tensor("dbg_gi", [128, 6, Bl, TP], BF16, kind="ExternalOutput")
        dbg["cT"] = nc.dram_tensor("dbg_cT", [128, Bl, 2, T], BF16, kind="ExternalOutput")
        dbg["acc_lse"] = nc.dram_tensor("dbg_acc_lse", [128, K], F32, kind="ExternalOutput")
        dbg["acc_dot"] = nc.dram_tensor("dbg_acc_dot", [128, K], F32, kind="ExternalOutput")
        dbg["zsum"] = nc.dram_tensor("dbg_zsum", [128, Bl * K], F32, kind="ExternalOutput")

    # ---- dram I/O ----
    # All constants ride in two packed tensors (one DMA each): the cost of a
    # DMA is dominated by fixed per-transfer overhead on the shared queue.
    WCOLS = 256 + 768 + 1536 + K * 256 + 128
    FCOLS = 1 + 6 + 2 + K + K + K + K + 1
    d_xT = nc.dram_tensor("xT", [128, 2, Bl, T], BF16, kind="ExternalInput")
    d_wpack = nc.dram_tensor("wpack", [128, WCOLS], BF16, kind="ExternalInput")
    d_fpack = nc.dram_tensor("fpack", [128, FCOLS], F32, kind="ExternalInput")
    d_out = nc.dram_tensor("out", [1, 1], F32, kind="ExternalOutput")

    with tile.TileContext(nc) as tc:
        with (
            tc.tile_pool(name="const", bufs=1) as cpool,
            tc.tile_pool(name="acc", bufs=1) as apool,
            tc.tile_pool(name="big", bufs=1) as bigpool,
        ):
            t_wpack = cpool.tile([128, WCOLS], BF16)
            t_fpack = cpool.tile([128, FCOLS], F32)
            nc.sync.dma_start(t_wpack[:], d_wpack[:])
            nc.sync.dma_start(t_fpack[:], d_fpack[:])

            wp_ap = t_wpack[:]
            off = 0
            t_Wep = _view(wp_ap, [(128, 2), (1, 128)], off); off += 256
            t_Wgi = _view(wp_ap, [(128, 6), (1, 128)], off); off += 768
            t_Wh = _view(wp_ap, [(768, 2), (128, 6), (1, 128)], off); off += 1536
            t_Wp = _view(wp_ap, [(256, K), (128, 2), (1, 128)], off); off += K * 256
            t_ident = wp_ap[:, off : off + 128]; off += 128
            assert off == WCOLS

            fp = t_fpack[:]
            off = 0
            t_bep = fp[:, off : off + 1]; off += 1
            t_bgi = fp[:, off : off + 6]; off += 6
            t_bhnw = fp[:, off : off + 2]; off += 2
            t_bp = fp[:, off : off + K]; off += K
            t_acc0 = fp[:, off : off + K]; off += K
            t_sA = fp[:, off : off + K]; off += K
            t_sB = fp[:, off : off + K]; off += K
            t_ones = fp[:, off : off + 1]; off += 1
            assert off == FCOLS

            acc_lse = apool.tile([128, K], F32)
            acc_dot = apool.tile([128, K], F32)
            nc.vector.tensor_copy(acc_lse[:], t_acc0)
            nc.vector.memset(acc_dot[:], 0.0)

            # persistent activations
            t_xT = bigpool.tile([128, 2, Bl, T], BF16)
            t_zT = bigpool.tile([128, Bl, T], BF16)
            t_gi = bigpool.tile([128, 6, Bl, TP], BF16)
            t_cT = bigpool.tile([128, Bl, 2, T], BF16)
            t_zsum = bigpool.tile([128, Bl * K], F32)

            if W:
                nc.vector.memset(t_gi[:, :, :, 0:W], 0.0)

            # ---------------- frontend ----------------
            # CH = 1024 (two sequences per chunk); x DMA is sliced per chunk
            # so the encoder starts as soon as the first slice lands
            CH = CH_FE
            nch = NT // CH
            assert nch * CH == NT
            SPC = CH // T  # sequences per chunk
            for ch in range(nch):
                for kb in range(2):
                    nc.sync.dma_start(
                        t_xT[:, kb, SPC * ch : SPC * (ch + 1)],
                        d_xT[:, kb, SPC * ch : SPC * (ch + 1)],
                    )

            xT_flat = t_xT[:].rearrange("p k b t -> p k (b t)")
            zT_flat = t_zT[:].rearrange("p b t -> p (b t)")

            gi_ap = t_gi[:]

            def fe_copy(i, dst, ps, bias):
                # PSUM->SBUF (+bias) copies split 4:3 ACT:DVE (ACT's per-1024
                # copy is ~15% cheaper, so it takes the larger share)
                if i % 7 < 4:
                    nc.scalar.activation(dst, ps, AF.Identity, bias=bias)
                else:
                    nc.vector.tensor_scalar_add(dst, ps, bias)

            # matmuls stream 512-col chunks (walrus moving-operand limit) into
            # 1024-col PSUM tiles; the PSUM->SBUF copies then run at 1024 cols
            # to halve the per-instruction access bubbles.
            it = 0
            PCH = 2 * CH
            npch = NT // PCH
            with tc.tile_pool(name="fe_ps", bufs=4, space="PSUM") as feps:
                # z = x @ (W_enc W_proj) + (b_enc W_proj + b_proj): the
                # encoder/projection pair is folded into one matmul on the
                # host (exact in real arithmetic), deleting the encT stage
                for pc in range(npch):
                    ps = feps.tile([128, 2, CH], F32)
                    for half in range(2):
                        ch = 2 * pc + half
                        for kb in range(2):
                            nc.tensor.matmul(
                                ps[:, half, :], t_Wep[:, kb, :],
                                xT_flat[:, kb, bass.ts(ch, CH)],
                                start=(kb == 0), stop=(kb == 1),
                            )
                    fe_copy(it, zT_flat[:, bass.ts(pc, PCH)],
                            ps[:].rearrange("p a b -> p (a b)"), t_bep[:, 0:1])
                    it += 1
                # gi = z @ Wi + bi (transposed), written at t-offset W
                for m in range(6):
                    for pc in range(npch):
                        ps = feps.tile([128, 2, CH], F32)
                        for half in range(2):
                            ch = 2 * pc + half
                            nc.tensor.matmul(
                                ps[:, half, :], t_Wgi[:, m, :],
                                zT_flat[:, bass.ts(ch, CH)],
                                start=True, stop=True,
                            )
                        fe_copy(
                            it,
                            t_gi[:, m, 2 * SPC * pc : 2 * SPC * (pc + 1), W:],
                            ps[:].rearrange("p a b -> p (a b)"),
                            t_bgi[:, m : m + 1],
                        )
                        it += 1

            # ---------------- GRU scan (chunk-parallel) ----------------
            zero_state = bigpool.tile([128, 2, L], BF16)
            nc.vector.memset(zero_state[:], 0.0)

            def gi_view(s, mlo, nmt, b0, nb):
                # [128, nmt, C, nb] at t = c*S + s (padded coords)
                return _view(
                    gi_ap,
                    [(Bl * TP, nmt), (S, C), (TP, nb)],
                    mlo * Bl * TP + b0 * TP + s,
                )

            def cT_view(s, b0, nb):
                # [128, 2, C, nb] at t = c*S + s - W  (cT is b-major so the
                # per-group byte ranges are disjoint -> no false cross-group
                # dependency between a group's scan and another's loss reads)
                return _view(
                    t_cT[:],
                    [(T, 2), (S, C), (2 * T, nb)],
                    b0 * 2 * T + s - W,
                )

            def run_scan(ghps, gpool, b0, nb, pfx):
                L2 = C * nb
                zs = zero_state[:, :, 0:L2]

                def alloc_gh(s):
                    gh = (
                        ghps.tile([128, 2, L2], F32, tag=f"{pfx}gh_r",
                                  name=f"{pfx}gh_r_{s}"),
                        ghps.tile([128, 2, L2], F32, tag=f"{pfx}gh_z",
                                  name=f"{pfx}gh_z_{s}"),
                        ghps.tile([128, 2, L2], F32, tag=f"{pfx}gh_n",
                                  name=f"{pfx}gh_n_{s}"),
                    )
                    # gi preadds via identity matmul (start the r/z groups)
                    nc.tensor.matmul(
                        gh[0][:], t_ident[:], gi_view(s, 0, 2, b0, nb),
                        start=True, stop=False, skip_group_check=True,
                    )
                    nc.tensor.matmul(
                        gh[1][:], t_ident[:], gi_view(s, 2, 2, b0, nb),
                        start=True, stop=False, skip_group_check=True,
                    )
                    return gh

                def mm_pass(gh, src, first, last, ms=(0, 1, 2, 3, 4, 5)):
                    # r tiles first so sigmoid(r) can start earliest
                    for m in ms:
                        ghx, mi = (
                            (gh[0], m) if m < 2 else
                            (gh[1], m - 2) if m < 4 else
                            (gh[2], m - 4)
                        )
                        for kb in range(2):
                            nc.tensor.matmul(
                                ghx[:, mi, :], t_Wh[:, kb, m, :], src[:, kb, :],
                                start=(first and m == 4 and kb == 0),
                                stop=(last and kb == 1),
                                skip_group_check=True,
                            )

                aa_prev = zs
                h_prev = None
                # PE order per step: the aa pass (chain-critical, r tiles
                # first) goes FIRST, then next step's preadds + bzh pass are
                # pre-issued so they never block the next aa->sigmoid edge.
                gh_cur = alloc_gh(0)
                mm_pass(gh_cur, zs, first=True, last=False)
                # POOL pre-copies gi_n into a contiguous tile one step ahead
                # so nsm (on-chain DVE) gets the packed-bf16 2x mode
                gic_cur = gpool.tile([128, 2, L2], BF16, tag=f"{pfx}gic", name=f"{pfx}gic_0")
                nc.gpsimd.tensor_copy(gic_cur[:], gi_view(0, 4, 2, b0, nb))
                bzh_pend = None
                for s in range(NS):
                    gh_r, gh_z, gh_n = gh_cur
                    if bzh_pend is None:
                        mm_pass(gh_cur, aa_prev, first=False, last=True)
                    else:
                        # interleave by region: bzh-r, aa-r(stop), bzh-z,
                        # aa-z(stop), bzh-n, aa-n(stop) -- sigma_r waits on
                        # only the first 8 MMs instead of the full 24
                        for ms in ((0, 1), (2, 3), (4, 5)):
                            mm_pass(
                                gh_cur, bzh_pend, first=(ms == (4, 5)), last=False, ms=ms
                            )
                            mm_pass(gh_cur, aa_prev, first=False, last=True, ms=ms)
                    # sigmoid(r) -> SBUF (npd may read only one PSUM input)
                    sig_r = gpool.tile([128, 2, L2], F32, tag=f"{pfx}sig_r")
                    nc.scalar.activation(sig_r[:], gh_r[:], AF.Sigmoid)
                    # sigmoid(z) -> SBUF (POOL cannot read PSUM)
                    sig_z = gpool.tile([128, 2, L2], BF16, tag=f"{pfx}sig_z")
                    nc.scalar.activation(sig_z[:], gh_z[:], AF.Sigmoid)
                    # off-chain: bzh = z * h_prev on POOL; omz = 1 - z on DVE
                    # (POOL at 0.42 eff can only afford one op per step at C=32)
                    bzh = gpool.tile([128, 2, L2], BF16, tag=f"{pfx}bzh")
                    if h_prev is None:
                        nc.gpsimd.tensor_tensor(bzh[:], sig_z[:], zs, op=ALU.mult)
                    else:
                        nc.gpsimd.tensor_tensor(bzh[:], sig_z[:], h_prev, op=ALU.mult)
                    omz = gpool.tile([128, 2, L2], BF16, tag=f"{pfx}omz")
                    nc.vector.tensor_scalar(
                        omz[:], sig_z[:], -1.0, 1.0, op0=ALU.mult, op1=ALU.add
                    )
                    # pre-issue next step's preadds; its bzh MMs are
                    # interleaved with the aa MMs at the next iteration
                    if s + 1 < NS:
                        gh_next = alloc_gh(s + 1)
                        bzh_pend = bzh
                        gh_cur = gh_next
                        gic_next = gpool.tile(
                            [128, 2, L2], BF16, tag=f"{pfx}gic",
                            name=f"{pfx}gic_{s + 1}"
                        )
                        nc.gpsimd.tensor_copy(gic_next[:], gi_view(s + 1, 4, 2, b0, nb))
                    # on-chain: npd -> nsm -> tanh -> aa
                    npd = gpool.tile([128, 2, L2], BF16, tag=f"{pfx}npd")
                    if bhn_zero:
                        nc.vector.tensor_tensor(npd[:], sig_r[:], gh_n[:], op=ALU.mult)
                    else:
                        for kb in range(2):
                            nc.vector.scalar_tensor_tensor(
                                npd[:, kb, :], gh_n[:, kb, :], t_bhnw[:, kb : kb + 1],
                                sig_r[:, kb, :], op0=ALU.add, op1=ALU.mult,
                            )
                    nsm = gpool.tile([128, 2, L2], BF16, tag=f"{pfx}nsm")
                    nc.vector.tensor_tensor(nsm[:], npd[:], gic_cur[:], op=ALU.add)
                    if s + 1 < NS:
                        gic_cur = gic_next
                    nn_ = gpool.tile([128, 2, L2], BF16, tag=f"{pfx}nn")
                    nc.scalar.activation(nn_[:], nsm[:], AF.Tanh)
                    aa = gpool.tile([128, 2, L2], BF16, tag=f"{pfx}aa")
                    nc.vector.tensor_tensor(aa[:], omz[:], nn_[:], op=ALU.mult)
                    # off-chain (DVE, after aa): h = aa + bzh -> cT (scratch in warmup)
                    if s >= W:
                        h_dst = cT_view(s, b0, nb)
                        nc.vector.tensor_tensor(h_dst, aa[:], bzh[:], op=ALU.add)
                        h_prev = h_dst
                    else:
                        h_t = gpool.tile([128, 2, L2], BF16, tag=f"{pfx}hwm")
                        nc.vector.tensor_tensor(h_t[:], aa[:], bzh[:], op=ALU.add)
                        h_prev = h_t[:]
                    aa_prev = aa
                    # interleaved emission: zsum for one b per early scan
                    # step (depends only on zT; fills DVE/POOL idle slots)
                    if emit_zsum and 1 <= s <= Bl:
                        b = s - 1
                        nc.vector.tensor_reduce(
                            t_zsum[:, b * K : b * K + 1], t_zT[:, b, 1:T],
                            axis=AX.X, op=ALU.add,
                        )
                        for k in range(2, K + 1):
                            nc.gpsimd.tensor_tensor(
                                t_zsum[:, b * K + k - 1 : b * K + k],
                                t_zsum[:, b * K + k - 2 : b * K + k - 1],
                                t_zT[:, b, k - 1 : k],
                                op=ALU.subtract,
                            )

            with (
                tc.tile_pool(name="gh_ps", bufs=2, space="PSUM") as ghps,
                tc.tile_pool(name="gate", bufs=2) as gpool,
            ):
                emit_zsum = len(scan_groups) == 1
                for b0, nb in scan_groups:
                    run_scan(ghps, gpool, b0, nb, pfx=f"g{b0}_")

            # ---------------- logits / loss ----------------
            if not (len(scan_groups) == 1):
                for b in range(Bl):
                    nc.vector.tensor_reduce(
                        t_zsum[:, b * K : b * K + 1], t_zT[:, b, 1:T],
                        axis=AX.X, op=ALU.add,
                    )
                    for k in range(2, K + 1):
                        nc.gpsimd.tensor_tensor(
                            t_zsum[:, b * K + k - 1 : b * K + k],
                            t_zsum[:, b * K + k - 2 : b * K + k - 1],
                            t_zT[:, b, k - 1 : k],
                            op=ALU.subtract,
                        )

            # negmax/sumexp for every (k, b, mtile) go to persistent buffers;
            # the Ln + LSE assembly is batched at the end (one ACT table set
            # per phase - no exp<->ln table thrashing).
            n_pairs = _ceil_div(n_m, 2)
            nm_all = bigpool.tile([128, K, Bl, n_m], F32)
            se_all = bigpool.tile([128, K, Bl, n_m], F32)
            with (
                tc.tile_pool(name="pred_ps", bufs=1, space="PSUM") as predps,
                tc.tile_pool(name="lg_ps", bufs=3, space="PSUM") as lgps,
                tc.tile_pool(name="es_ps", bufs=1, space="PSUM") as esps,
                tc.tile_pool(name="pred_sb", bufs=1) as predsb,
                tc.tile_pool(name="small", bufs=3) as small,
            ):
                # exp output is never read; a single PSUM bank absorbs the
                # writes (ACT is in-order so reuse needs no sync)
                es = esps.tile([128, 512], F32)
                # explicit double-buffer for pred_sb so the padding tail
                # (cols >= Tk) can be zeroed once per slot: k runs descending
                # so Tk grows and never exposes stale data in the tail.
                pred_tiles = [
                    predsb.tile([128, n_m * 128], BF16, tag=f"pt{i}", name=f"pred_sb{i}")
                    for i in range(2)
                ]
                it = 0
                for gi_g, (gb0, gnb) in enumerate(scan_groups):
                  for pt in pred_tiles:
                    nc.vector.memset(pt[:, T - K :], 0.0)
                  for k in range(K, 0, -1):
                    Tk = T - k
                    for b in range(gb0, gb0 + gnb):
                        predp = predps.tile([128, 512], F32)
                        for hb in range(2):
                            nc.tensor.matmul(
                                predp[:, 0:Tk], t_Wp[:, k - 1, hb, :], t_cT[:, b, hb, 0:Tk],
                                start=(hb == 0), stop=(hb == 1),
                            )
                        preds = pred_tiles[it % 2]
                        it += 1
                        psum_t = small.tile([128, 1], F32, tag="predsum")
                        nc.vector.tensor_scalar(
                            preds[:, 0:Tk], predp[:, 0:Tk],
                            t_bp[:, k - 1 : k], None, op0=ALU.add, op1=ALU.add,
                            accum_out=psum_t[:],
                        )
                        prod = small.tile([128, 1], F32, tag="prod")
                        nc.gpsimd.tensor_tensor(
                            prod[:], psum_t[:], t_zsum[:, b * K + k - 1 : b * K + k],
                            op=ALU.mult,
                        )
                        nc.gpsimd.tensor_tensor(
                            acc_dot[:, k - 1 : k], acc_dot[:, k - 1 : k], prod[:], op=ALU.add
                        )
                        for pair in range(n_pairs):
                            mlo = pair * 2
                            mhi = min(mlo + 2, n_m)
                            nmt = mhi - mlo
                            lg = lgps.tile([128, 2, 512], F32)
                            for mi in range(nmt):
                                nc.tensor.matmul(
                                    lg[:, mi, 0:Tk],
                                    preds[:, bass.ts(mlo + mi, 128)],
                                    t_zT[:, b, k:T],
                                    start=True, stop=True,
                                )
                            # exact row max (POOL cannot read PSUM -> DVE only)
                            nc.vector.tensor_reduce(
                                nm_all[:, k - 1, b, mlo:mhi], lg[:, 0:nmt, 0:Tk],
                                axis=AX.X, op=ALU.max, negate=True,
                            )
                            for mi in range(nmt):
                                nc.scalar.activation(
                                    es[:, 0:Tk], lg[:, mi, 0:Tk], AF.Exp,
                                    bias=nm_all[:, k - 1, b, mlo + mi : mlo + mi + 1],
                                    accum_out=se_all[:, k - 1, b, mlo + mi : mlo + mi + 1],
                                )

                # batched LSE assembly + final reduction to scalar
                lse_all = bigpool.tile([128, K, Bl, n_m], F32)
                nc.scalar.activation(lse_all[:], se_all[:], AF.Ln)
                lsf_all = bigpool.tile([128, K, Bl, n_m], F32)
                nc.vector.tensor_tensor(lsf_all[:], lse_all[:], nm_all[:], op=ALU.subtract)
                lred = small.tile([128, K], F32, tag="lred")
                nc.vector.tensor_reduce(
                    lred[:], lsf_all[:].rearrange("p k b m -> p k (b m)"),
                    axis=AX.X, op=ALU.add,
                )
                nc.vector.tensor_tensor(acc_lse[:], acc_lse[:], lred[:], op=ALU.add)
                t1 = small.tile([128, K], F32, tag="t1")
                nc.vector.tensor_tensor(t1[:], acc_lse[:], t_sA[:], op=ALU.mult)
                t2 = small.tile([128, K], F32, tag="t2")
                nc.vector.tensor_tensor(t2[:], acc_dot[:], t_sB[:], op=ALU.mult)
                t3 = small.tile([128, K], F32, tag="t3")
                nc.vector.tensor_tensor(t3[:], t1[:], t2[:], op=ALU.subtract)
                red = small.tile([128, 1], F32, tag="redf")
                nc.vector.tensor_reduce(red[:], t3[:], axis=AX.X, op=ALU.add)
                # final scalar reduction reuses the es scratch bank (all
                # exps are done by now; WAW ordering is tracked)
                fin = es[0:1, 0:1]
                nc.tensor.matmul(fin, t_ones[:], red[:], start=True, stop=True)
                outsb = small.tile([1, 1], F32, tag="outsb")
                nc.vector.tensor_copy(outsb[:], fin)
                nc.sync.dma_start(d_out[:], outsb[:])

                if debug:
                    nc.sync.dma_start(dbg["zT"][:], t_zT[:])
                    nc.sync.dma_start(dbg["gi"][:], t_gi[:])
                    nc.sync.dma_start(dbg["cT"][:], t_cT[:])
                    nc.sync.dma_start(dbg["acc_lse"][:], acc_lse[:])
                    nc.sync.dma_start(dbg["acc_dot"][:], acc_dot[:])
                    nc.sync.dma_start(dbg["zsum"][:], t_zsum[:])

    nc.compile()
    return nc


def prepare_inputs(inputs, Bl, T, K):
    """Host-side: shard + layout transform. Returns list of in_maps (per core)."""
    x = np.asarray(inputs["x_seq"], np.float32)
    W_enc = np.asarray(inputs["W_enc"], np.float32)
    b_enc = np.asarray(inputs["b_enc"], np.float32)
    W_proj = np.asarray(inputs["W_proj"], np.float32)
    b_proj = np.asarray(inputs["b_proj"], np.float32)
    Wi = np.asarray(inputs["Wi"], np.float32)
    bi = np.asarray(inputs["bi"], np.float32)
    Wh = np.asarray(inputs["Wh"], np.float32)
    bhn = np.asarray(inputs["bhn"], np.float32)
    Wp = np.asarray(inputs["Wp"], np.float32)[:K] / np.float32(TEMP)
    bp = np.asarray(inputs["bp"], np.float32)[:K] / np.float32(TEMP)

    B = x.shape[0]
    n_cores = B // Bl
    n_m = _ceil_div(T - 1, 128)

    W_ep = (W_enc.astype(np.float64) @ W_proj.astype(np.float64)).astype(np.float32)
    b_ep = (b_enc.astype(np.float64) @ W_proj.astype(np.float64)
            + b_proj.astype(np.float64)).astype(np.float32)
    Wep_c = np.ascontiguousarray(
        W_ep.reshape(2, 128, 128).transpose(1, 0, 2)
    ).reshape(128, 256)
    Wgi_c = np.ascontiguousarray(Wi.reshape(128, 6, 128)).reshape(128, 768)
    Wh_c = np.ascontiguousarray(
        Wh.reshape(2, 128, 6, 128).transpose(1, 0, 2, 3)
    ).reshape(128, 1536)
    Wp_c = np.ascontiguousarray(
        Wp.reshape(K, 2, 128, 128).transpose(2, 0, 1, 3)
    ).reshape(128, K * 256)
    ident = np.eye(128, dtype=np.float32)

    acc0 = np.zeros((128, K), np.float64)
    sA = np.zeros((128, K), np.float64)
    sB = np.zeros((128, K), np.float64)
    for k in range(1, K + 1):
        Tk = T - k
        sA[:, k - 1] = 1.0 / (K * B * Tk)
        sB[:, k - 1] = 1.0 / (K * B * Tk * Tk)
        rem = Tk - (n_m - 1) * 128  # valid rows in last mtile
        if rem < 128:
            acc0[rem:, k - 1] = -Bl * math.log(Tk)

    common = {}
    common["wpack"] = np.concatenate(
        [Wep_c, Wgi_c, Wh_c, Wp_c, ident], axis=1
    ).astype(nbf)
    common["fpack"] = np.concatenate(
        [
            b_ep.reshape(128, 1),
            np.ascontiguousarray(bi.reshape(6, 128).T),
            np.ascontiguousarray(bhn.reshape(2, 128).T),
            np.ascontiguousarray(bp.T),
            acc0, sA, sB,
            np.ones((128, 1)),
        ],
        axis=1,
    ).astype(np.float32)

    in_maps = []
    for c in range(n_cores):
        shard = x[c * Bl : (c + 1) * Bl]  # [Bl, T, F]
        xT = np.ascontiguousarray(shard.transpose(2, 0, 1)).astype(nbf)  # [F, Bl, T]
        xT = np.ascontiguousarray(
            xT.reshape(2, 128, Bl, T).transpose(1, 0, 2, 3)
        )  # [128, 2, Bl, T]
        m = dict(common)
        m["xT"] = xT
        in_maps.append(m)
    return in_maps


_CACHE = {}


def _get_built(Bl, T, K, C=32, W=4, bhn_zero=True, debug=False):
    key = (Bl, T, K, C, W, bhn_zero, debug)
    if key not in _CACHE:
        _CACHE[key] = build_kernel(Bl, T, K, C=C, W=W, bhn_zero=bhn_zero, debug=debug)
    return _CACHE[key]


def run(inputs, Bl=8, T=T_FULL, K=K_FULL, n_cores=N_CORES, trace=False,
        C=32, W=4, debug=False):
    bhn_zero = not np.any(np.asarray(inputs["bhn"]))
    nc = _get_built(Bl, T, K, C=C, W=W, bhn_zero=bool(bhn_zero), debug=debug)
    in_maps = prepare_inputs(inputs, Bl, T, K)[:n_cores]
    res = run_bass_kernel_spmd(nc, in_maps, core_ids=list(range(len(in_maps))), trace=trace)
    partials = [r["out"][0, 0] for r in res.results]
    loss = np.float32(np.sum(np.asarray(partials, np.float32)))
    return loss, res


def kernel(**inputs) -> np.ndarray:
    loss, _ = run(inputs)
    return np.asarray(loss, np.float32)
